# revision 1
# baseline (speedup 1.0000x reference)
"""ConvIntNet (interaction-network) Trainium2 kernel.

Strategy (pure data parallelism over batch, 8 cores x 16 batch elements):
  The dense one-hot relation einsums are algebraically removed. With edges
  ordered receiver-major, edge (r, s) has
      h1 = relu(A[r] + S[s] + eb1),  A = xn @ W1_rec, S = xn @ W1_snd
  so stage 1 is a broadcast-add + relu (per-partition-scalar ops), stages
  2/3 are block-diagonal-packed matmuls, and the receiver scatter-add is a
  segmented sum over s fused into the stage-3 relu via accum_out. Self-edge
  (s == r) contributions are computed by a small diagonal pipeline and
  subtracted. BatchNorm is folded into W1/biases on the host.

Layout per batch element:
  nodes padded 150 -> 168 = 4 groups x 42; partition dim carries
  4 x (30|15|6)-feature groups; free dim carries (q, s) edge positions in
  14 chunks of 450 = 3 q-blocks x 150 senders.
"""

import numpy as np

import concourse.bacc as bacc
import concourse.tile as tile
from concourse import mybir

f32 = mybir.dt.float32
bf16 = mybir.dt.bfloat16
Alu = mybir.AluOpType
Act = mybir.ActivationFunctionType
AxX = mybir.AxisListType.X

# ---- problem dims (hardcoded per contract) ----
B, N, F = 128, 150, 16
NCORES = 8
BL = B // NCORES          # 16 batch elements per core
EH, EH2, NEFF = 30, 15, 6
DH, DH2, NDYN = 45, 22, 6
ABS, NCLS = 48, 5
BN_EPS = 1e-3
NP = 168                  # padded nodes = 4 * 42
QG = NP // 4              # 42 q positions per partition group
NCH = 14                  # chunks of 450 = 3 q-blocks
CHW = 450

# h1 engine split: q -> engine
H1_DVE = set(range(0, 24))
H1_GP = set(range(24, 38))
H1_ACT = set(range(38, 42))


def _build_module():
    nc = bacc.Bacc("TRN2", target_bir_lowering=False)

    xt_d = nc.dram_tensor("xt", [BL, F, NP], f32, kind="ExternalInput")
    wr_d = nc.dram_tensor("wr", [F, 32], bf16, kind="ExternalInput")
    ws_d = nc.dram_tensor("ws", [F, 128], bf16, kind="ExternalInput")
    w2_d = nc.dram_tensor("w2", [128, 64], bf16, kind="ExternalInput")
    w3_d = nc.dram_tensor("w3", [128, 64], bf16, kind="ExternalInput")
    w3s_d = nc.dram_tensor("w3s", [64, 32], bf16, kind="ExternalInput")
    zst_d = nc.dram_tensor("zst", [24, 4 * DH], f32, kind="ExternalInput")
    w1x_d = nc.dram_tensor("w1x", [F, DH], f32, kind="ExternalInput")
    wd2_d = nc.dram_tensor("wd2", [DH, DH2], f32, kind="ExternalInput")
    wd3_d = nc.dram_tensor("wd3", [DH2, NDYN], f32, kind="ExternalInput")
    wa1_d = nc.dram_tensor("wa1", [NDYN, ABS], f32, kind="ExternalInput")
    wa2_d = nc.dram_tensor("wa2", [ABS + 1, NCLS], f32, kind="ExternalInput")
    bia_d = nc.dram_tensor("bia", [128, 11], f32, kind="ExternalInput")
    bab_d = nc.dram_tensor("bab", [128, 2], f32, kind="ExternalInput")
    ones_d = nc.dram_tensor("ones", [1, BL], f32, kind="ExternalInput")
    y_d = nc.dram_tensor("y", [BL, NCLS], f32, kind="ExternalOutput")

    from contextlib import ExitStack
    ctx = ExitStack()
    with tile.TileContext(nc) as tc, ctx:
        consts = ctx.enter_context(tc.tile_pool(name="consts", bufs=1))
        xp = ctx.enter_context(tc.tile_pool(name="xp", bufs=2))
        asp = ctx.enter_context(tc.tile_pool(name="asp", bufs=2))
        h1p = ctx.enter_context(tc.tile_pool(name="h1p", bufs=2))
        h2p = ctx.enter_context(tc.tile_pool(name="h2p", bufs=3))
        effp = ctx.enter_context(tc.tile_pool(name="effp", bufs=3))
        rp = ctx.enter_context(tc.tile_pool(name="rp", bufs=8))
        smp = ctx.enter_context(tc.tile_pool(name="smp", bufs=3))
        psAS = ctx.enter_context(tc.tile_pool(name="psAS", bufs=2, space="PSUM"))
        ps2 = ctx.enter_context(tc.tile_pool(name="ps2", bufs=2, space="PSUM"))
        ps3 = ctx.enter_context(tc.tile_pool(name="ps3", bufs=2, space="PSUM"))
        psD = ctx.enter_context(tc.tile_pool(name="psD", bufs=2, space="PSUM"))

        # ---- load constants ----
        wr_t = consts.tile([F, 32], bf16)
        ws_t = consts.tile([F, 128], bf16)
        w2_t = consts.tile([128, 64], bf16)
        w3_t = consts.tile([128, 64], bf16)
        w3s_t = consts.tile([64, 32], bf16)
        zst_t = consts.tile([24, 4 * DH], f32)
        w1x_t = consts.tile([F, DH], f32)
        wd2_t = consts.tile([DH, DH2], f32)
        wd3_t = consts.tile([DH2, NDYN], f32)
        wa1_t = consts.tile([NDYN, ABS], f32)
        wa2_t = consts.tile([ABS + 1, NCLS], f32)
        bia_t = consts.tile([128, 11], f32)
        bab_t = consts.tile([128, 2], f32)
        for t, d in [(wr_t, wr_d), (ws_t, ws_d), (w2_t, w2_d), (w3_t, w3_d),
                     (w3s_t, w3s_d), (zst_t, zst_d), (w1x_t, w1x_d),
                     (wd2_t, wd2_d), (wd3_t, wd3_d), (wa1_t, wa1_d),
                     (wa2_t, wa2_d), (bia_t, bia_d), (bab_t, bab_d)]:
            nc.sync.dma_start(out=t, in_=d.ap())

        bA = bab_t[:, 0:1]          # bf16 A-side bias (incl eb1), 4x30 pattern
        bS = bab_t[:, 1:2]          # bf16 S-side bias
        eb2r = bia_t[:, 2:3]        # fp32, rows 64u+15j+f2
        eb3r = bia_t[:, 3:4]        # fp32, rows 64g+32u+6j+c
        eb3d = bia_t[0:32, 4:5]
        db1 = bia_t[0:DH, 5:6]
        db2 = bia_t[0:DH2, 6:7]
        db3 = bia_t[0:NDYN, 7:8]
        ab1 = bia_t[0:ABS, 8:9]
        sc_pp = bia_t[0:F, 9:10]
        sh_pp = bia_t[0:F, 10:11]

        pooled = consts.tile([NDYN, BL], f32)

        for b in range(BL):
            x_t = xp.tile([F, NP], f32, tag="x_t")
            nc.sync.dma_start(out=x_t, in_=xt_d.ap()[b])

            x_tb = xp.tile([F, NP], bf16, tag="x_tb")
            nc.vector.tensor_copy(out=x_tb, in_=x_t)

            # A[32j+f, q] = xn[42j+q] @ W1r'  (4 matmuls, M=32 col-tiles)
            a_ps_full = psAS.tile([128, 512], f32, tag="as")
            a_ps = a_ps_full[:, 0:QG]
            for j in range(4):
                nc.tensor.matmul(
                    out=a_ps[32 * j:32 * j + 32, :], lhsT=wr_t,
                    rhs=x_tb[:, QG * j:QG * (j + 1)],
                    start=True, stop=True, tile_position=(0, 32 * j))
            a_s = asp.tile([128, QG], f32, tag="a_s")
            nc.scalar.activation(out=a_s, in_=a_ps, func=Act.Identity, bias=bA)
            a_sb = asp.tile([128, QG], bf16, tag="a_sb")
            nc.vector.tensor_copy(out=a_sb, in_=a_s)

            # S_rep[32j+f, s] = xn[s] @ W1s' (one matmul, 4x replicated lhsT)
            s_ps_full = psAS.tile([128, 512], f32, tag="as")
            s_ps = s_ps_full[:, 0:NP]
            nc.tensor.matmul(out=s_ps, lhsT=ws_t, rhs=x_tb, start=True, stop=True)
            s_rep = asp.tile([128, NP], bf16, tag="s_rep")
            nc.scalar.activation(out=s_rep, in_=s_ps, func=Act.Identity, bias=bS)

            # h1[p, q*150+s] = relu(S_rep[p, s] + A[p, q])
            h1 = h1p.tile([128, NCH * CHW], bf16, tag="h1")
            for q in range(QG):
                dst = h1[:, q * N:(q + 1) * N]
                if q in H1_DVE:
                    nc.vector.tensor_scalar(
                        out=dst, in0=s_rep[:, 0:N], scalar1=a_s[:, q:q + 1],
                        scalar2=0.0, op0=Alu.add, op1=Alu.max)
                elif q in H1_GP:
                    nc.gpsimd.tensor_scalar(
                        out=dst, in0=s_rep[:, 0:N], scalar1=a_s[:, q:q + 1],
                        scalar2=0.0, op0=Alu.add, op1=Alu.max)
                else:
                    nc.scalar.activation(
                        out=dst, in_=s_rep[:, 0:N], func=Act.Relu,
                        bias=a_s[:, q:q + 1])

            # stage 2 + 3 + fused relu/segment-sum
            r_tiles = []
            p3_cur = None
            h2_cur = None
            p2_cur = None
            for c in range(NCH):
                half = c % 2
                if half == 0:
                    p2_full = ps2.tile([128, 512], f32, tag="p2")
                    p2_cur = p2_full[:, 0:CHW]
                nc.tensor.matmul(
                    out=p2_cur[64 * half:64 * half + 64, :], lhsT=w2_t,
                    rhs=h1[:, c * CHW:(c + 1) * CHW],
                    start=True, stop=True, tile_position=(0, 64 * half))
                if half == 1 or c == NCH - 1:
                    p = c // 2
                    rows = 128 if half == 1 else 64
                    h2_cur = h2p.tile([128, CHW], bf16, tag="h2")
                    nc.vector.tensor_scalar(
                        out=h2_cur[0:rows, 0:225], in0=p2_cur[0:rows, 0:225],
                        scalar1=eb2r[0:rows], scalar2=0.0,
                        op0=Alu.add, op1=Alu.max)
                    nc.scalar.activation(
                        out=h2_cur[0:rows, 225:450], in_=p2_cur[0:rows, 225:450],
                        func=Act.Relu, bias=eb2r[0:rows])
                    g = p % 2
                    if g == 0:
                        p3_full = ps3.tile([128, 512], f32, tag="p3")
                        p3_cur = p3_full[:, 0:CHW]
                        r_t = rp.tile([128, 3], f32, tag="r")
                        r_tiles.append(r_t)
                    nc.tensor.matmul(
                        out=p3_cur[64 * g:64 * g + 64, 0:CHW], lhsT=w3_t,
                        rhs=h2_cur, start=True, stop=True,
                        tile_position=(0, 64 * g))
                    if g == 1 or p == 6:
                        rows3 = 128 if g == 1 else 64
                        r_t = r_tiles[-1]
                        for i in range(3):
                            scr = effp.tile([128, N], bf16, tag="escr")
                            src = p3_cur[0:rows3, i * N:(i + 1) * N]
                            if i % 2 == 0:
                                nc.vector.tensor_scalar(
                                    out=scr[0:rows3, :], in0=src,
                                    scalar1=eb3r[0:rows3], scalar2=0.0,
                                    op0=Alu.add, op1=Alu.max,
                                    accum_out=r_t[0:rows3, i:i + 1])
                            else:
                                nc.scalar.activation(
                                    out=scr[0:rows3, :], in_=src, func=Act.Relu,
                                    bias=eb3r[0:rows3],
                                    accum_out=r_t[0:rows3, i:i + 1])

            # gather segment sums -> EffR[6j+c', q]
            effr = smp.tile([24, QG], f32, tag="effr")
            for c in range(NCH):
                w_, g_, u_ = c // 4, (c % 4) // 2, c % 2
                nc.sync.dma_start(
                    out=effr[:, 3 * c:3 * c + 3],
                    in_=r_tiles[w_][64 * g_ + 32 * u_:64 * g_ + 32 * u_ + 24, :])

            # diagonal (self-edge) pipeline
            pd = smp.tile([128, QG], bf16, tag="pd")
            for j in range(4):
                nc.vector.tensor_tensor(
                    out=pd[32 * j:32 * (j + 1), :],
                    in0=a_sb[32 * j:32 * (j + 1), :],
                    in1=s_rep[32 * j:32 * (j + 1), QG * j:QG * (j + 1)],
                    op=Alu.add)
            pdr = smp.tile([128, QG], bf16, tag="pdr")
            nc.vector.tensor_scalar_max(out=pdr, in0=pd, scalar1=0.0)
            pd2_full = psD.tile([64, 512], f32, tag="dps")
            pd2 = pd2_full[:, 0:QG]
            nc.tensor.matmul(out=pd2, lhsT=w2_t, rhs=pdr, start=True, stop=True)
            h2d = smp.tile([64, QG], bf16, tag="h2d")
            nc.scalar.activation(out=h2d, in_=pd2, func=Act.Relu, bias=eb2r[0:64])
            pd3_full = psD.tile([32, 512], f32, tag="dps")
            pd3 = pd3_full[:, 0:QG]
            nc.tensor.matmul(out=pd3, lhsT=w3s_t, rhs=h2d, start=True, stop=True)
            eself = smp.tile([32, QG], f32, tag="eself")
            nc.scalar.activation(out=eself, in_=pd3, func=Act.Relu, bias=eb3d)

            effrf = smp.tile([24, QG], f32, tag="effrf")
            nc.vector.tensor_tensor(out=effrf, in0=effr, in1=eself[0:24, :],
                                    op=Alu.subtract)

            # dynamics MLP; node sum fused into last relu
            xn_t = xp.tile([F, NP], f32, tag="xn_t")
            nc.vector.tensor_scalar(out=xn_t, in0=x_t, scalar1=sc_pp,
                                    scalar2=sh_pp, op0=Alu.mult, op1=Alu.add)
            d1a_full = psD.tile([DH, 512], f32, tag="dps")
            d1a = d1a_full[:, 0:N]
            nc.tensor.matmul(out=d1a, lhsT=w1x_t, rhs=xn_t[:, 0:N],
                             start=True, stop=True)
            d1e_full = psD.tile([DH, 512], f32, tag="dps")
            d1e = d1e_full[:, 0:N]
            for j in range(4):
                cnt = min(QG, N - QG * j)
                nc.tensor.matmul(
                    out=d1e[:, QG * j:QG * j + cnt],
                    lhsT=zst_t[:, DH * j:DH * (j + 1)],
                    rhs=effrf[:, 0:cnt], start=True, stop=True)
            d1es = smp.tile([DH, N], f32, tag="d1es")
            nc.vector.tensor_scalar(out=d1es, in0=d1e, scalar1=db1,
                                    scalar2=None, op0=Alu.add)
            d1sum = smp.tile([DH, N], f32, tag="d1sum")
            nc.vector.tensor_tensor(out=d1sum, in0=d1a, in1=d1es, op=Alu.add)
            d1s = smp.tile([DH, N], f32, tag="d1s")
            nc.scalar.activation(out=d1s, in_=d1sum, func=Act.Relu)
            d2_full = psD.tile([DH2, 512], f32, tag="dps")
            d2 = d2_full[:, 0:N]
            nc.tensor.matmul(out=d2, lhsT=wd2_t, rhs=d1s, start=True, stop=True)
            d2s = smp.tile([DH2, N], f32, tag="d2s")
            nc.scalar.activation(out=d2s, in_=d2, func=Act.Relu, bias=db2)
            d3_full = psD.tile([NDYN, 512], f32, tag="dps")
            d3 = d3_full[:, 0:N]
            nc.tensor.matmul(out=d3, lhsT=wd3_t, rhs=d2s, start=True, stop=True)
            dyn_s = smp.tile([NDYN, N], f32, tag="dyn_s")
            nc.scalar.activation(out=dyn_s, in_=d3, func=Act.Relu, bias=db3,
                                 accum_out=pooled[:, b:b + 1])

        # abstract MLP + softmax (once per core)
        pa_full = psD.tile([ABS, 512], f32, tag="dps")
        pa = pa_full[:, 0:BL]
        nc.tensor.matmul(out=pa, lhsT=wa1_t, rhs=pooled, start=True, stop=True)
        ha = consts.tile([ABS + 1, BL], f32)
        nc.scalar.activation(out=ha[0:ABS, :], in_=pa, func=Act.Relu, bias=ab1)
        nc.sync.dma_start(out=ha[ABS:ABS + 1, :], in_=ones_d.ap())
        zl_full = psD.tile([BL, 512], f32, tag="dps")
        zl = zl_full[:, 0:NCLS]
        nc.tensor.matmul(out=zl, lhsT=ha, rhs=wa2_t, start=True, stop=True)
        ex = consts.tile([BL, NCLS], f32)
        nc.scalar.activation(out=ex, in_=zl, func=Act.Exp)
        ssum = consts.tile([BL, 1], f32)
        nc.vector.tensor_reduce(out=ssum, in_=ex, axis=AxX, op=Alu.add)
        rcp = consts.tile([BL, 1], f32)
        nc.vector.reciprocal(out=rcp, in_=ssum)
        outt = consts.tile([BL, NCLS], f32)
        nc.vector.tensor_scalar_mul(out=outt, in0=ex, scalar1=rcp)
        nc.sync.dma_start(out=y_d.ap(), in_=outt)

    nc.compile()
    return nc


def _prep_consts(inp):
    """Host-side weight preprocessing (tiny, O(KB))."""
    g = lambda k: np.asarray(inp[k], np.float32)
    sc = g("bn_gamma") / np.sqrt(g("bn_var") + BN_EPS)
    sh = g("bn_beta") - g("bn_mean") * sc
    W1 = g("eW1")
    W1r = sc[:, None] * W1[:F]
    W1s = sc[:, None] * W1[F:]
    bA = sh @ W1[:F] + g("eb1")
    bS = sh @ W1[F:]

    wr = np.zeros((F, 32), np.float32)
    wr[:, :EH] = W1r
    ws = np.zeros((F, 128), np.float32)
    for j in range(4):
        ws[:, 32 * j:32 * j + EH] = W1s

    w2 = np.zeros((128, 64), np.float32)
    eW2 = g("eW2")
    for j in range(4):
        w2[32 * j:32 * j + EH, 15 * j:15 * j + EH2] = eW2
    w3 = np.zeros((128, 64), np.float32)
    eW3 = g("eW3")
    for u in range(2):
        for j in range(4):
            w3[64 * u + 15 * j:64 * u + 15 * j + EH2,
               32 * u + 6 * j:32 * u + 6 * j + NEFF] = eW3
    w3s = np.zeros((64, 32), np.float32)
    for j in range(4):
        w3s[15 * j:15 * j + EH2, 6 * j:6 * j + NEFF] = eW3

    dW1 = g("dW1")
    zst = np.zeros((24, 4 * DH), np.float32)
    for j in range(4):
        zst[6 * j:6 * j + NEFF, DH * j:DH * (j + 1)] = dW1[F:F + NEFF]

    wa2 = np.vstack([g("aW2"), g("ab2")[None, :]]).astype(np.float32)

    bia = np.zeros((128, 11), np.float32)
    bab = np.zeros((128, 2), np.float32)
    for j in range(4):
        bab[32 * j:32 * j + EH, 0] = bA
        bab[32 * j:32 * j + EH, 1] = bS
        bia[15 * j:15 * j + EH2, 2] = g("eb2")
        bia[64 + 15 * j:64 + 15 * j + EH2, 2] = g("eb2")
        bia[6 * j:6 * j + NEFF, 4] = g("eb3")
        for gg in range(2):
            for u in range(2):
                bia[64 * gg + 32 * u + 6 * j:64 * gg + 32 * u + 6 * j + NEFF,
                    3] = g("eb3")
    bia[0:DH, 5] = g("db1")
    bia[0:DH2, 6] = g("db2")
    bia[0:NDYN, 7] = g("db3")
    bia[0:ABS, 8] = g("ab1")
    bia[0:F, 9] = sc
    bia[0:F, 10] = sh

    import ml_dtypes
    tobf = lambda a: np.asarray(a, np.float32).astype(ml_dtypes.bfloat16)
    return {
        "wr": tobf(wr), "ws": tobf(ws), "w2": tobf(w2), "w3": tobf(w3),
        "w3s": tobf(w3s), "zst": zst, "w1x": dW1[:F].astype(np.float32),
        "wd2": g("dW2"), "wd3": g("dW3"), "wa1": g("aW1"), "wa2": wa2,
        "bia": bia, "bab": bab, "ones": np.ones((1, BL), np.float32),
    }


def _prep_xt(x):
    """x (B, N, F) -> per-core transposed/padded (NCORES, BL, F, NP)."""
    x = np.asarray(x, np.float32)
    xt = np.zeros((B, F, NP), np.float32)
    xt[:, :, :N] = np.transpose(x, (0, 2, 1))
    return xt.reshape(NCORES, BL, F, NP)


_NC_CACHE = {}


def _get_module():
    if "nc" not in _NC_CACHE:
        _NC_CACHE["nc"] = _build_module()
    return _NC_CACHE["nc"]


def make_in_maps(inputs):
    consts = _prep_consts(inputs)
    xt = _prep_xt(inputs["x"])
    return [dict(consts, xt=np.ascontiguousarray(xt[c])) for c in range(NCORES)]


def kernel(**inputs) -> np.ndarray:
    from concourse.bass_utils import run_bass_kernel_spmd
    nc = _get_module()
    in_maps = make_in_maps(inputs)
    res = run_bass_kernel_spmd(nc, in_maps, core_ids=list(range(NCORES)))
    return np.concatenate([r["y"] for r in res.results], axis=0)



# revision 10
# speedup vs baseline: 497.4843x; 497.4843x over previous
"""ConvIntNet (interaction-network) Trainium2 kernel.

Strategy: pure data parallelism over batch (8 cores x 16 elements). The
dense one-hot relation einsums are removed algebraically: with edges
receiver-major, edge (r, s) has h1 = relu(A[r] + S[s] + b), where
A = xn @ W1_rec and S = xn @ W1_snd, so stage 1 is a broadcast add
(single DVE op via stride-0 access patterns), stages 2/3 are
block-diagonal-packed matmuls, and the receiver scatter-add is a
segmented sum fused into the stage-3 relu via accum_out. Self-edges
(s == r) are recomputed by a small diagonal pipeline and subtracted.
BatchNorm is folded into W1/biases on the host.

The per-call dispatch cost on this runtime is dominated by emitted
instruction count (BIR recompile + NEFF load per call), so the batch
loop is a hardware For_i with statically allocated tiles (~90 emitted
instructions total), all weights are baked into the NEFF as Const
tensors (module cached per weight hash), and x is uploaded as fp16.
"""

import hashlib
import numpy as np

import concourse.bacc as bacc
import concourse.tile as tile
from concourse import mybir

f32 = mybir.dt.float32
f16 = mybir.dt.float16
Alu = mybir.AluOpType
Act = mybir.ActivationFunctionType
AxX = mybir.AxisListType.X

# ---- problem dims (hardcoded per contract) ----
B, N, F = 128, 150, 16
NCORES = 8
BL = B // NCORES          # 16 batch elements per core
EH, EH2, NEFF = 30, 15, 6
DH, DH2, NDYN = 45, 22, 6
ABS, NCLS = 48, 5
BN_EPS = 1e-3
NP = 168                  # padded nodes = 4 groups x 42
QG = NP // 4              # 42 receiver positions per partition group
NCH = 14                  # h1 chunks of 450 = 3 q-positions x 150 senders
CHW = 450

# const blob column offsets
C16_WR, C16_WS, C16_WRS, C16_W2, C16_W3, C16_W3S = 0, 32, 160, 192, 256, 320
C16_COLS = 352
C32_ZST, C32_W1X, C32_WD2, C32_WD3, C32_WA1, C32_WA2, C32_BIA = (
    0, 180, 225, 247, 253, 301, 306)
C32_COLS = 318
# bias column indices within C32_BIA
B_A, B_S, B_AS, B_E2, B_E3, B_E3D, B_D1, B_D2, B_D3, B_AB1, B_SC, B_SH = range(12)


def _prep_consts(inp):
    """Host-side weight preprocessing -> two const blobs (tiny, O(100KB))."""
    g = lambda k: np.asarray(inp[k], np.float32)
    sc = g("bn_gamma") / np.sqrt(g("bn_var") + BN_EPS)
    sh = g("bn_beta") - g("bn_mean") * sc
    W1 = g("eW1")
    W1r = sc[:, None] * W1[:F]
    W1s = sc[:, None] * W1[F:]
    bA = sh @ W1[:F] + g("eb1")
    bS = sh @ W1[F:]

    c16 = np.zeros((128, C16_COLS), np.float32)
    c16[:F, C16_WR:C16_WR + EH] = W1r
    for j in range(4):
        c16[:F, C16_WS + 32 * j:C16_WS + 32 * j + EH] = W1s
    c16[:F, C16_WRS:C16_WRS + EH] = W1r + W1s
    eW2, eW3 = g("eW2"), g("eW3")
    for j in range(4):
        c16[32 * j:32 * j + EH, C16_W2 + 15 * j:C16_W2 + 15 * j + EH2] = eW2
        c16[15 * j:15 * j + EH2, C16_W3S + 6 * j:C16_W3S + 6 * j + NEFF] = eW3
        for u in range(2):
            c16[64 * u + 15 * j:64 * u + 15 * j + EH2,
                C16_W3 + 32 * u + 6 * j:C16_W3 + 32 * u + 6 * j + NEFF] = eW3

    c32 = np.zeros((128, C32_COLS), np.float32)
    dW1 = g("dW1")
    for j in range(4):
        c32[6 * j:6 * j + NEFF, C32_ZST + DH * j:C32_ZST + DH * (j + 1)] = \
            dW1[F:F + NEFF]
    c32[:F, C32_W1X:C32_W1X + DH] = dW1[:F]
    c32[:DH, C32_WD2:C32_WD2 + DH2] = g("dW2")
    c32[:DH2, C32_WD3:C32_WD3 + NDYN] = g("dW3")
    c32[:NDYN, C32_WA1:C32_WA1 + ABS] = g("aW1")
    c32[:ABS, C32_WA2:C32_WA2 + NCLS] = g("aW2")
    c32[ABS, C32_WA2:C32_WA2 + NCLS] = g("ab2")
    bia = np.zeros((128, 12), np.float32)
    for j in range(4):
        bia[32 * j:32 * j + EH, B_A] = bA
        bia[32 * j:32 * j + EH, B_S] = bS
        bia[32 * j:32 * j + EH, B_AS] = bA + bS
        bia[15 * j:15 * j + EH2, B_E2] = g("eb2")
        bia[64 + 15 * j:64 + 15 * j + EH2, B_E2] = g("eb2")
        bia[6 * j:6 * j + NEFF, B_E3D] = g("eb3")
        for gg in range(2):
            for u in range(2):
                bia[64 * gg + 32 * u + 6 * j:64 * gg + 32 * u + 6 * j + NEFF,
                    B_E3] = g("eb3")
    bia[:DH, B_D1] = g("db1")
    bia[:DH2, B_D2] = g("db2")
    bia[:NDYN, B_D3] = g("db3")
    bia[:ABS, B_AB1] = g("ab1")
    bia[:F, B_SC] = sc
    bia[:F, B_SH] = sh
    c32[:, C32_BIA:] = bia
    return c16.astype(np.float16), c32


def _build_module(c16, c32):
    nc = bacc.Bacc("TRN2", target_bir_lowering=False)

    xt_d = nc.dram_tensor("xt", [BL, F, N], f16, kind="ExternalInput")
    y_d = nc.dram_tensor("y", [BL, NCLS], f32, kind="ExternalOutput")
    pool_d = nc.dram_tensor("pool_scr", [BL, NDYN], f32, kind="Internal")
    c16_d = nc.inline_tensor(c16, name="c16")
    c32_d = nc.inline_tensor(c32, name="c32")
    ones_d = nc.inline_tensor(np.ones((1, BL), np.float32), name="ones_c")

    with tile.TileContext(nc) as tc:
        with tc.tile_pool(name="sb", bufs=1) as sb, \
             tc.tile_pool(name="ps", bufs=1, space="PSUM") as psp:
            cb16 = sb.tile([128, C16_COLS], f16)
            cb32 = sb.tile([128, C32_COLS], f32)
            wr = cb16[0:F, C16_WR:C16_WR + 32]
            ws = cb16[0:F, C16_WS:C16_WS + 128]
            wrs = cb16[0:F, C16_WRS:C16_WRS + 32]
            w2 = cb16[:, C16_W2:C16_W2 + 64]
            w3 = cb16[:, C16_W3:C16_W3 + 64]
            w3s = cb16[0:64, C16_W3S:C16_W3S + 32]
            bia = cb32[:, C32_BIA:]
            bcol = lambda k, r=128: bia[0:r, k:k + 1]

            x_t = sb.tile([F, NP], f16)
            a_s = sb.tile([128, QG], f16)
            s_rep = sb.tile([128, N], f16)
            h1p = sb.tile([128, NCH * CHW], f16)
            h1 = sb.tile([128, NCH * CHW], f16)
            h2 = [sb.tile([128, CHW], f16, name=f"h2_{k}") for k in range(2)]
            scr = [sb.tile([128, N], f16, name=f"scr_{k}") for k in range(2)]
            r_all = sb.tile([128, 12], f32)
            eff48 = sb.tile([24, 48], f32)
            pdr = sb.tile([128, QG], f16)
            h2d = sb.tile([64, QG], f16)
            eself = sb.tile([32, QG], f32)
            effrf = sb.tile([24, QG], f32)
            xn_t = sb.tile([F, N], f32)
            d1s = sb.tile([DH, N], f32)
            d2s = sb.tile([DH2, N], f32)
            dyn_scr = sb.tile([NDYN, N], f32)
            dsum = sb.tile([NDYN, 1], f32)
            pooled = sb.tile([NDYN, BL], f32)
            ha = sb.tile([ABS + 1, BL], f32)
            ex = sb.tile([BL, NCLS], f32)
            ssum = sb.tile([BL, 1], f32)
            rcp = sb.tile([BL, 1], f32)
            outt = sb.tile([BL, NCLS], f32)

            asb = psp.tile([128, 512], f32)   # a: [:,0:42], s: [:,64:214]
            p2 = [psp.tile([128, 512], f32, name=f"p2_{k}") for k in range(2)]
            p3 = psp.tile([128, 512], f32)
            psD = psp.tile([128, 512], f32)   # diag: 0:42 / 64:106 / 128:170
            d1 = psp.tile([DH, 512], f32)
            d23 = psp.tile([128, 512], f32)   # d2 0:22 / d3 64:70 / pa / zl

            nc.sync.dma_start(out=cb16, in_=c16_d.ap())
            nc.sync.dma_start(out=cb32, in_=c32_d.ap())
            nc.vector.memset(x_t, 0.0)
            nc.sync.dma_start(out=ha[ABS:ABS + 1, :], in_=ones_d.ap())

            with tc.For_i(0, BL, 1) as i:
                nc.sync.dma_start(out=x_t[:, 0:N], in_=xt_d.ap()[i])

                # A[32j+f, q] = xn[42j+q] @ W1r'; S[32j+f, s] = xn[s] @ W1s'
                a_ps = asb[:, 0:QG]
                for j in range(4):
                    nc.tensor.matmul(
                        out=a_ps[32 * j:32 * j + 32, :], lhsT=wr,
                        rhs=x_t[:, QG * j:QG * (j + 1)],
                        start=True, stop=True, tile_position=(0, 32 * j))
                nc.scalar.activation(out=a_s, in_=a_ps, func=Act.Identity,
                                     bias=bcol(B_A))
                s_ps = asb[:, 64:64 + N]
                nc.tensor.matmul(out=s_ps, lhsT=ws, rhs=x_t[:, 0:N],
                                 start=True, stop=True)
                nc.scalar.activation(out=s_rep, in_=s_ps, func=Act.Identity,
                                     bias=bcol(B_S))

                # h1[p, q*150+s] = relu(A[p, q] + S[p, s]) via stride-0 APs
                nc.vector.tensor_tensor(
                    out=h1p.rearrange("p (q s) -> p q s", q=QG),
                    in0=a_s.unsqueeze(2).broadcast_to([128, QG, N]),
                    in1=s_rep.unsqueeze(1).broadcast_to([128, QG, N]),
                    op=Alu.add)
                nc.scalar.activation(out=h1, in_=h1p, func=Act.Relu)

                # stage 2 + 3 + fused relu/segment-sum
                for c in range(NCH):
                    half = c % 2
                    p2c = p2[(c // 2) % 2][:, 0:CHW]
                    nc.tensor.matmul(
                        out=p2c[64 * half:64 * half + 64, :], lhsT=w2,
                        rhs=h1[:, c * CHW:(c + 1) * CHW],
                        start=True, stop=True, tile_position=(0, 64 * half))
                    if half == 1:
                        p = c // 2
                        h2c = h2[p % 2]
                        nc.scalar.activation(out=h2c, in_=p2c, func=Act.Relu,
                                             bias=bcol(B_E2))
                        g = p % 2
                        nc.tensor.matmul(
                            out=p3[64 * g:64 * g + 64, 0:CHW], lhsT=w3,
                            rhs=h2c, start=True, stop=True,
                            tile_position=(0, 64 * g))
                        if g == 1 or p == 6:
                            rows = 128 if g == 1 else 64
                            t = p // 2
                            for i3 in range(3):
                                src = p3[0:rows, i3 * N:(i3 + 1) * N]
                                acc = r_all[0:rows, 3 * t + i3:3 * t + i3 + 1]
                                if i3 % 2 == 0:
                                    nc.vector.tensor_scalar(
                                        out=scr[0][0:rows, :], in0=src,
                                        scalar1=bcol(B_E3, rows), scalar2=0.0,
                                        op0=Alu.add, op1=Alu.max,
                                        accum_out=acc)
                                else:
                                    nc.scalar.activation(
                                        out=scr[1][0:rows, :], in_=src,
                                        func=Act.Relu, bias=bcol(B_E3, rows),
                                        accum_out=acc)

                # gather segment sums -> eff48[6j+c', 12t+6g+3u+i3]
                eff3 = eff48.rearrange("r (t x) -> r t x", t=4)
                for gg in range(2):
                    for u in range(2):
                        nt = 4 if gg == 0 else 3
                        rb = 64 * gg + 32 * u
                        nc.sync.dma_start(
                            out=eff3[:, 0:nt, 6 * gg + 3 * u:6 * gg + 3 * u + 3],
                            in_=r_all[rb:rb + 24].rearrange(
                                "r (t i) -> r t i", t=4)[:, 0:nt, :])

                # diagonal (self-edge) pipeline: recompute and subtract
                pd_ps = psD[:, 0:QG]
                for j in range(4):
                    nc.tensor.matmul(
                        out=pd_ps[32 * j:32 * j + 32, :], lhsT=wrs,
                        rhs=x_t[:, QG * j:QG * (j + 1)],
                        start=True, stop=True, tile_position=(0, 32 * j))
                nc.scalar.activation(out=pdr, in_=pd_ps, func=Act.Relu,
                                     bias=bcol(B_AS))
                nc.tensor.matmul(out=psD[0:64, 64:64 + QG], lhsT=w2, rhs=pdr,
                                 start=True, stop=True)
                nc.scalar.activation(out=h2d, in_=psD[0:64, 64:64 + QG],
                                     func=Act.Relu, bias=bcol(B_E2, 64))
                nc.tensor.matmul(out=psD[0:32, 128:128 + QG], lhsT=w3s,
                                 rhs=h2d, start=True, stop=True)
                nc.scalar.activation(out=eself, in_=psD[0:32, 128:128 + QG],
                                     func=Act.Relu, bias=bcol(B_E3D, 32))
                nc.vector.tensor_tensor(out=effrf, in0=eff48[:, 0:QG],
                                        in1=eself[0:24, :], op=Alu.subtract)

                # dynamics MLP; d1 = W1x'@xn + Wie'@eff accumulated in PSUM
                nc.vector.tensor_scalar(out=xn_t, in0=x_t[:, 0:N],
                                        scalar1=bcol(B_SC, F),
                                        scalar2=bcol(B_SH, F),
                                        op0=Alu.mult, op1=Alu.add)
                d1c = d1[:, 0:N]
                nc.tensor.matmul(out=d1c, lhsT=cb32[0:F, C32_W1X:C32_W1X + DH],
                                 rhs=xn_t, start=True, stop=False)
                for j in range(4):
                    cnt = min(QG, N - QG * j)
                    nc.tensor.matmul(
                        out=d1c[:, QG * j:QG * j + cnt],
                        lhsT=cb32[0:24, C32_ZST + DH * j:C32_ZST + DH * (j + 1)],
                        rhs=effrf[:, 0:cnt], start=False, stop=(j == 3))
                nc.scalar.activation(out=d1s, in_=d1c, func=Act.Relu,
                                     bias=bcol(B_D1, DH))
                nc.tensor.matmul(out=d23[0:DH2, 0:N],
                                 lhsT=cb32[0:DH, C32_WD2:C32_WD2 + DH2],
                                 rhs=d1s, start=True, stop=True)
                nc.scalar.activation(out=d2s, in_=d23[0:DH2, 0:N],
                                     func=Act.Relu, bias=bcol(B_D2, DH2))
                nc.tensor.matmul(out=d23[64:64 + NDYN, 0:N],
                                 lhsT=cb32[0:DH2, C32_WD3:C32_WD3 + NDYN],
                                 rhs=d2s, start=True, stop=True)
                nc.scalar.activation(out=dyn_scr, in_=d23[64:64 + NDYN, 0:N],
                                     func=Act.Relu, bias=bcol(B_D3, NDYN),
                                     accum_out=dsum)
                nc.sync.dma_start(out=pool_d.ap()[i], in_=dsum)

            # abstract MLP + softmax (once per core)
            nc.sync.dma_start(out=pooled, in_=pool_d.ap().rearrange("b d -> d b"))
            nc.tensor.matmul(out=d23[0:ABS, 160:160 + BL],
                             lhsT=cb32[0:NDYN, C32_WA1:C32_WA1 + ABS],
                             rhs=pooled, start=True, stop=True)
            nc.scalar.activation(out=ha[0:ABS, :], in_=d23[0:ABS, 160:160 + BL],
                                 func=Act.Relu, bias=bcol(B_AB1, ABS))
            nc.tensor.matmul(out=d23[64:64 + BL, 200:200 + NCLS], lhsT=ha,
                             rhs=cb32[0:ABS + 1, C32_WA2:C32_WA2 + NCLS],
                             start=True, stop=True)
            nc.scalar.activation(out=ex, in_=d23[64:64 + BL, 200:200 + NCLS],
                                 func=Act.Exp)
            nc.vector.tensor_reduce(out=ssum, in_=ex, axis=AxX, op=Alu.add)
            nc.vector.reciprocal(out=rcp, in_=ssum)
            nc.vector.tensor_scalar_mul(out=outt, in0=ex, scalar1=rcp)
            nc.sync.dma_start(out=y_d.ap(), in_=outt)

    nc.compile()
    return nc


_NC_CACHE = {}
_WKEYS = ("bn_gamma", "bn_beta", "bn_mean", "bn_var", "eW1", "eb1", "eW2",
          "eb2", "eW3", "eb3", "dW1", "db1", "dW2", "db2", "dW3", "db3",
          "aW1", "ab1", "aW2", "ab2")


def _get_module(inputs):
    h = hashlib.sha256()
    for k in _WKEYS:
        h.update(np.ascontiguousarray(np.asarray(inputs[k], np.float32)))
    key = h.hexdigest()
    if key not in _NC_CACHE:
        c16, c32 = _prep_consts(inputs)
        _NC_CACHE[key] = _build_module(c16, c32)
    return _NC_CACHE[key]


def make_in_maps(inputs):
    """x (B, N, F) f32 -> per-core transposed (BL, F, N) f16."""
    x = np.asarray(inputs["x"], np.float32)
    xt = np.ascontiguousarray(np.transpose(x, (0, 2, 1))).astype(np.float16)
    xt = xt.reshape(NCORES, BL, F, N)
    return [{"xt": np.ascontiguousarray(xt[c])} for c in range(NCORES)]


def kernel(**inputs) -> np.ndarray:
    from concourse.bass_utils import run_bass_kernel_spmd
    nc = _get_module(inputs)
    in_maps = make_in_maps(inputs)
    res = run_bass_kernel_spmd(nc, in_maps, core_ids=list(range(NCORES)))
    return np.concatenate([r["y"] for r in res.results], axis=0)
